# revision 51
# baseline (speedup 1.0000x reference)
"""
Bass/Trainium2 kernel for nn_BottleneckShared (moe_routing).

Computation (per sample b):
    rw   = sigmoid(mean_hw(x) @ router_w.T + router_b)          # [E]
    Wk_b = sum_e rw[e] * wk[e]            (k = 1,2,3)           # per-sample conv kernels
    out  = relu(bn3(conv3(relu(bn2(conv2(relu(bn1(conv1(x)))))))) + x)

Sharding: data-parallel over batch. 64 samples -> 8 NeuronCores x 8 samples.
Expert banks / router / BN params are replicated to every core.

Device-side design notes:
 - BN scales are folded into the expert weight banks on the host; BN biases
   are applied in the ScalarE (ACT) drain of each conv's PSUM accumulation.
 - Per-sample weight combination runs on the TensorEngine via a
   scaled-identity trick: matmul(out, lhsT = rw[b,e] * I128, rhs = bank[e])
   accumulated over e in PSUM produces sum_e rw[e]*bank[e] directly in the
   [contraction, out-channel] layout the conv matmuls need.
 - conv1/conv3 are 1x1 convs = plain matmuls over the 784 pixels.
   conv2 (3x3, pad 1) = 9 accumulating matmuls with shifted access patterns
   over a zero-padded [128, 30, 30] input tile.
 - All matmul operands are fp16 (PE runs 16-bit at 1 cycle/row vs 4 for
   fp32; fp16's 10-bit mantissa keeps rel-err ~1e-3); PSUM accumulation is
   fp32 as always.
 - Emission is a rolling pipeline: routers for samples 0..2 up front, then
   each sample's combine+convs emits the router of sample s+3 behind it.
   This interleaves the in-order DVE queue (router reductions vs residual
   FMAs) so no engine queue holds cross-sample hazards.
 - DMA issue order is hand-scheduled (the DMA device serves in issue order
   and each dma_start costs a flat descriptor-generation slot): sample 0's
   x tiles, then expert banks split into per-combine-chunk tiles, with
   later samples' x loads threaded between bank transfers.
"""

import sys

import numpy as np

sys.path.insert(0, "/opt/trn_rl_repo")

import concourse.bacc as bacc
import concourse.bass as bass
import concourse.mybir as mybir
import concourse.tile as tile
from concourse import bass_utils

EPS = 1e-5

B = 64          # global batch
NCORES = 8
BS = B // NCORES  # samples per core
E = 8           # experts
CIN = 512
WID = 128       # bottleneck width
COUT = 512
H = 28
P = H * H       # 784 pixels
NCH = 392       # pixels per conv output chunk (14 rows)

F16 = mybir.dt.float16
F32 = mybir.dt.float32


def build_program():
    nc = bacc.Bacc("TRN2", target_bir_lowering=False, debug=False)

    # ---- DRAM I/O (per-core shapes) ----
    x_d = nc.dram_tensor("x", [BS, 4, 128, P], F16, kind="ExternalInput")
    b1_d = nc.dram_tensor("bank1", [128, E * 512], F16, kind="ExternalInput")
    b2_d = nc.dram_tensor("bank2", [128, E * 1152], F16, kind="ExternalInput")
    b3_d = nc.dram_tensor("bank3", [128, E * 512], F16, kind="ExternalInput")
    cc_d = nc.dram_tensor("cc", [128, 168], F16, kind="ExternalInput")
    bias1_d = nc.dram_tensor("bias1", [128, 1], F32, kind="ExternalInput")
    bias2_d = nc.dram_tensor("bias2", [128, 1], F32, kind="ExternalInput")
    bias3_d = nc.dram_tensor("bias3", [128, 4], F32, kind="ExternalInput")
    out_d = nc.dram_tensor("out", [BS, 4, 128, P], F16, kind="ExternalOutput")

    Relu = mybir.ActivationFunctionType.Relu
    Sigmoid = mybir.ActivationFunctionType.Sigmoid
    Copy = mybir.ActivationFunctionType.Copy
    ADD = mybir.AluOpType.add

    with tile.TileContext(nc) as tc:
        with (
            tc.tile_pool(name="const", bufs=1) as constp,
            tc.tile_pool(name="xin", bufs=6) as xp,
            tc.tile_pool(name="xsplit", bufs=1) as xsp,
            tc.tile_pool(name="ids", bufs=4) as idsp,
            tc.tile_pool(name="comb", bufs=2) as combp,
            tc.tile_pool(name="act", bufs=4) as actp,
            tc.tile_pool(name="small", bufs=3) as smallp,
            tc.tile_pool(name="scratch", bufs=1) as scratchp,
            tc.tile_pool(name="rwb", bufs=BS) as rwbp,
            tc.tile_pool(name="resid", bufs=4) as residp,
            tc.tile_pool(name="pscomb", bufs=3, space=bass.MemorySpace.PSUM) as pscombp,
            tc.tile_pool(name="psconv", bufs=4, space=bass.MemorySpace.PSUM) as psconvp,
            tc.tile_pool(name="psr", bufs=1, space=bass.MemorySpace.PSUM) as psrp,
        ):
            # ---- persistent constants ----
            # Banks go over gpsimd/SWDGE so they don't serialize behind the
            # per-sample x loads on the sync/HWDGE queue.
            bank1a = constp.tile([128, 4 * 512], F16)  # experts 0-3, col=(e*4+it)*128+o
            bank1b = constp.tile([128, 4 * 512], F16)  # experts 4-7
            b2c1 = constp.tile([128, E * 512], F16)    # w2 cols 0:512 per expert
            b2c2 = constp.tile([128, E * 512], F16)    # w2 cols 512:1024
            b2c3 = constp.tile([128, E * 128], F16)    # w2 cols 1024:1152
            bank3 = constp.tile([128, E * 512], F16)   # col = e*512 + o
            cc = constp.tile([128, 168], F16)  # [rwt(32) | ident(128) | rb row0]
            rwt = cc[:, 0:32]
            ident = cc[:, 32:160]
            rb = cc[0:1, 160:168]
            ones1 = constp.tile([1, 1], F16)
            bias1 = constp.tile([128, 1], F32)
            bias2 = constp.tile([128, 1], F32)
            bias3 = constp.tile([128, 4], F32)

            nc.gpsimd.memset(ones1[:], 1.0)

            # DMA issue order = device service order in the cost model.
            # Sample 0's dependencies go first (x0 split per tile so the
            # router reduce can start after ~1us), then the small consts,
            # then banks in combine-chunk order, then the remaining x.
            xs_l, ids_l, rwb_l = [], [], []
            for s in (0, 1):
                xs_l.append([
                    xsp.tile([128, P], F16, tag=f"xs{s}_{t}", name=f"xs{s}_{t}")
                    for t in range(4)
                ])
            xs0 = xs_l[0]
            for s in range(2, BS):
                big = xp.tile([128, 4 * P], F16, tag="xs", name=f"xs{s}")
                xs_l.append([big[:, t * P : (t + 1) * P] for t in range(4)])
            nc.sync.dma_start(xs0[0][:], x_d[0, 0])
            nc.sync.dma_start(xs0[1][:], x_d[0, 1])
            nc.sync.dma_start(xs0[2][:], x_d[0, 2])
            nc.sync.dma_start(xs0[3][:], x_d[0, 3])
            nc.sync.dma_start(bank1a[:], b1_d[:, : 4 * 512])
            nc.sync.dma_start(cc[:], cc_d[:])
            nc.sync.dma_start(bank1b[:], b1_d[:, 4 * 512 :])
            nc.sync.dma_start(bank3[:], b3_d[:])
            nc.sync.dma_start(xs_l[1][0][:], x_d[1, 0])
            nc.sync.dma_start(xs_l[1][1][:], x_d[1, 1])
            nc.sync.dma_start(b2c1[:], b2_d[:, 0 : E * 512])
            nc.sync.dma_start(xs_l[1][2][:], x_d[1, 2])
            nc.sync.dma_start(xs_l[1][3][:], x_d[1, 3])
            nc.sync.dma_start(bias1[:], bias1_d[:])
            nc.sync.dma_start(bias2[:], bias2_d[:])
            nc.sync.dma_start(bias3[:], bias3_d[:])
            nc.sync.dma_start(b2c2[:], b2_d[:, E * 512 : 2 * E * 512])
            nc.sync.dma_start(b2c3[:], b2_d[:, 2 * E * 512 :])
            for s in range(2, BS):
                nc.sync.dma_start(
                    xs_l[s][0].tensor[:, :], x_d[s].transpose([1, 0, 2])
                )

            # ================= routers =====================================
            def emit_router(s):
                xs = xs_l[s]

                pooled = smallp.tile([128, 4], F32, tag="pooled")
                ndve = 2 if s == 0 else 4
                for t in range(ndve):
                    nc.vector.tensor_reduce(
                        pooled[:, t : t + 1],
                        xs[t][:, :],
                        axis=mybir.AxisListType.X,
                        op=ADD,
                    )
                for t in range(ndve, 4):
                    # sample 0 only: run half the pooling on ACT (accum_out)
                    # to shorten the kernel-front critical path.
                    scratch = scratchp.tile([128, P], F16, tag="scratch")
                    nc.scalar.activation(
                        scratch[:],
                        xs[t][:, :],
                        Copy,
                        accum_out=pooled[:, t : t + 1],
                    )
                pooled16 = smallp.tile([128, 4], F16, tag="pooled16")
                nc.vector.tensor_copy(pooled16[:], pooled[:])

                rpsum = psrp.tile([128, E], F32, tag="rpsum")
                for t in range(4):
                    nc.tensor.matmul(
                        rpsum[:],
                        pooled16[:, t : t + 1].broadcast_to([128, 128]),
                        rwt[:, t * E : (t + 1) * E],
                        start=(t == 0),
                        stop=False,
                    )
                nc.tensor.matmul(
                    rpsum[:],
                    ones1[:].broadcast_to([1, 128]),
                    rb,
                    start=False,
                    stop=True,
                )
                rwb = rwbp.tile([128, E], F32, tag="rwb", name=f"rwb{s}")
                nc.scalar.activation(rwb[:], rpsum[:], Sigmoid)
                rwb_l.append(rwb)

                ids = idsp.tile([128, E * 128], F16, tag="ids", name=f"ids{s}")
                for e in range(E):
                    nc.vector.tensor_scalar_mul(
                        ids[:, e * 128 : (e + 1) * 128], ident, rwb[:, e : e + 1]
                    )
                ids_l.append(ids)

            # Rolling emission: routers for samples 0..2 up front, then each
            # sample's compute emits the router of sample s+3 behind it so the
            # DVE queue interleaves router reduces with residual STTs.
            for s in range(3):
                emit_router(s)

            # ============ phase B: combine + convs, per sample ==============
            for s in range(BS):
                if s + 3 < BS:
                    emit_router(s + 3)
                xs = xs_l[s]
                ids = ids_l[s]
                rwb7 = rwb_l[s][:, 7:8].tensor[:, 7:8]

                # ---- combine per-sample weights on PE (chunk-major) ----
                w1c = combp.tile([128, 512], F16, tag="w1c")    # [i, (it,o)]
                w2c = combp.tile([128, 1152], F16, tag="w2c")   # [ci, (tap,o)]
                w3c = combp.tile([128, 512], F16, tag="w3c")    # [ci, o]
                def combine(bk_of, wid, dst, d0):
                    psc = pscombp.tile([128, 512], F32, tag="psc", name="psc")
                    for e in range(E):
                        nc.tensor.matmul(
                            psc[:, :wid],
                            ids[:, e * 128 : (e + 1) * 128],
                            bk_of(e),
                            start=(e == 0),
                            stop=(e == E - 1),
                        )
                    nc.scalar.activation(dst[:, d0 : d0 + wid], psc[:, :wid], Copy)

                def half_bank(a, b, wid):
                    return lambda e: (a if e < 4 else b)[
                        :, (e % 4) * wid : (e % 4 + 1) * wid
                    ]

                def flat_bank(t, wid):
                    return lambda e: t[:, e * wid : (e + 1) * wid]

                combine(half_bank(bank1a, bank1b, 512), 512, w1c, 0)
                combine(flat_bank(bank3, 512), 512, w3c, 0)

                # ---- conv1 (1x1) + bn1 + relu -> padded mid1 [128, 30, 30] ----
                mid1 = actp.tile([128, 30, 30], F16, tag="mid1")
                nc.gpsimd.memset(mid1[:], 0.0)
                for c in range(2):
                    ps1 = psconvp.tile([128, 14, 28], F32, tag="convps")
                    for k in range(4):
                        nc.tensor.matmul(
                            ps1[:],
                            w1c[:, k * 128 : (k + 1) * 128],
                            xs[k][:, c * NCH : (c + 1) * NCH],
                            start=(k == 0),
                            stop=(k == 3),
                        )
                    nc.scalar.activation(
                        mid1[:, 14 * c + 1 : 14 * c + 15, 1:29],
                        ps1[:],
                        Relu,
                        bias=bias1[:],
                    )

                combine(flat_bank(b2c1, 512), 512, w2c, 0)
                combine(flat_bank(b2c2, 512), 512, w2c, 512)
                combine(flat_bank(b2c3, 128), 128, w2c, 1024)

                # ---- conv2 (3x3, pad 1) + bn2 + relu -> out2 [128, 784] ----
                out2 = actp.tile([128, P], F16, tag="out2")
                for c in range(2):
                    ps2 = psconvp.tile([128, 14, 28], F32, tag="convps")
                    idx = 0
                    for dy in range(3):
                        for dx in range(3):
                            nc.tensor.matmul(
                                ps2[:],
                                w2c[:, (dy * 3 + dx) * 128 : (dy * 3 + dx + 1) * 128],
                                mid1[:, 14 * c + dy : 14 * c + dy + 14, dx : dx + 28],
                                start=(idx == 0),
                                stop=(idx == 8),
                            )
                            idx += 1
                    nc.scalar.activation(
                        out2[:, c * NCH : (c + 1) * NCH], ps2[:], Relu, bias=bias2[:]
                    )

                # ---- conv3 (1x1) + bn3 + residual + relu -> ofull ----
                ofull = actp.tile([128, 4 * P], F16, tag="ofull")
                for m in range(4):
                    for c in range(2):
                        ps3 = psconvp.tile([128, 14, 28], F32, tag="convps")
                        nc.tensor.matmul(
                            ps3[:],
                            w3c[:, m * 128 : (m + 1) * 128],
                            out2[:, c * NCH : (c + 1) * NCH],
                            start=True,
                            stop=True,
                        )
                        u = residp.tile([128, NCH], F32, tag="u")
                        nc.vector.scalar_tensor_tensor(
                            u[:],
                            xs[m][:, c * NCH : (c + 1) * NCH],
                            bias3[:, m : m + 1],
                            ps3[:].rearrange("p a b -> p (a b)"),
                            op0=ADD,
                            op1=ADD,
                        )
                        nc.gpsimd.tensor_scalar_max(
                            ofull[:, m * P + c * NCH : m * P + (c + 1) * NCH],
                            u[:],
                            0.0,
                        )
                    nc.sync.dma_start(
                        out_d[s, m], ofull[:, m * P : (m + 1) * P]
                    )

    nc.compile()
    return nc


_NC_CACHE = None


def _get_program():
    global _NC_CACHE
    if _NC_CACHE is None:
        _NC_CACHE = build_program()
    return _NC_CACHE


def prepare_inputs(
    x, router_w, router_b, w1, w2, w3,
    g1, b1, m1, v1, g2, b2, m2, v2, g3, b3, m3, v3,
):
    """Host-side preprocessing -> per-core in_maps."""
    f = np.float32
    x = np.asarray(x, f)
    router_w = np.asarray(router_w, f)
    router_b = np.asarray(router_b, f)
    w1 = np.asarray(w1, f)
    w2 = np.asarray(w2, f)
    w3 = np.asarray(w3, f)

    s1 = np.asarray(g1, f) / np.sqrt(np.asarray(v1, f) + EPS)
    s2 = np.asarray(g2, f) / np.sqrt(np.asarray(v2, f) + EPS)
    s3 = np.asarray(g3, f) / np.sqrt(np.asarray(v3, f) + EPS)
    bb1 = np.asarray(b1, f) - np.asarray(m1, f) * s1
    bb2 = np.asarray(b2, f) - np.asarray(m2, f) * s2
    bb3 = np.asarray(b3, f) - np.asarray(m3, f) * s3

    # bank1: [E, o=128, i=512] * s1[o] -> [E, i, o] -> [E, 4, 128, 128]
    w1s = w1[:, :, :, 0, 0] * s1[None, :, None]
    # [E, o, i] -> [E, it, p, o] -> sbuf rows p, cols (e, it, o)
    bank1 = np.ascontiguousarray(
        w1s.transpose(0, 2, 1).reshape(E, 4, 128, 128).transpose(2, 0, 1, 3)
        .reshape(128, E * 512)
    ).astype(np.float16)
    # bank2: [E, o, ci, dy, dx] * s2[o] -> [E, tap, ci, o] -> rows ci
    w2s = w2 * s2[None, :, None, None, None]
    b2flat = (
        w2s.transpose(0, 3, 4, 2, 1).reshape(E, 9, 128, 128).transpose(2, 0, 1, 3)
        .reshape(128, E, 1152)
    ).astype(np.float16)
    bank2 = np.ascontiguousarray(np.concatenate(
        [
            b2flat[:, :, 0:512].reshape(128, E * 512),
            b2flat[:, :, 512:1024].reshape(128, E * 512),
            b2flat[:, :, 1024:1152].reshape(128, E * 128),
        ],
        axis=1,
    ))
    # bank3: [E, o=512, ci=128] * s3[o] -> [E, ci, o] -> rows ci
    w3s = w3[:, :, :, 0, 0] * s3[None, :, None]
    bank3 = np.ascontiguousarray(
        w3s.transpose(0, 2, 1).transpose(1, 0, 2).reshape(128, E * 512)
    ).astype(np.float16)

    rwt = np.ascontiguousarray(
        (router_w / float(P)).T.reshape(4, 128, E)
    ).astype(np.float16)
    cc = np.zeros((128, 168), np.float16)
    cc[:, 0:32] = rwt.transpose(1, 0, 2).reshape(128, 32)
    cc[:, 32:160] = np.eye(128, dtype=np.float16)
    cc[0, 160:168] = router_b.astype(np.float16)
    bias1 = bb1.reshape(128, 1)
    bias2 = bb2.reshape(128, 1)
    bias3 = np.ascontiguousarray(bb3.reshape(4, 128).T)

    x16 = x.reshape(B, 4, 128, P).astype(np.float16)

    shared = {
        "bank1": bank1,
        "bank2": bank2,
        "bank3": bank3,
        "cc": cc,

        "bias1": bias1,
        "bias2": bias2,
        "bias3": bias3,
    }
    in_maps = []
    for c in range(NCORES):
        m = dict(shared)
        m["x"] = np.ascontiguousarray(x16[c * BS : (c + 1) * BS])
        in_maps.append(m)
    return in_maps


def run(in_maps, trace=False, tmpdir=None):
    nc = _get_program()
    res = bass_utils.run_bass_kernel_spmd(
        nc, in_maps, core_ids=list(range(NCORES)), trace=trace, tmpdir=tmpdir
    )
    outs = [np.asarray(r["out"], np.float32) for r in res.results]
    full = np.concatenate(outs, axis=0).reshape(B, CIN, H, H)
    return full, res


def kernel(**inputs):
    in_maps = prepare_inputs(**inputs)
    full, _ = run(in_maps, trace=False)
    return full
